# revision 19
# baseline (speedup 1.0000x reference)
"""DendriticMLP Trainium2 kernel — 8-core hybrid data/tensor parallel.

v3 vs baseline (pure data-parallel): the dendrite path (89% of matmul
cycles, 402 MB/core of fp32r segment-weight streaming when replicated) is
tensor-sharded: core c computes d = ctx @ sw for unit quarter r = c % 4
(512 units) over batch group q = c // 4 (2048 rows). Segment-weight traffic
drops to ~120 MB/core (~170 MB total vs 433 MB), removing the HBM
co-binding that made replicated runs stochastically DMA-bound (baseline
measured 1.85-2.10 ms run-to-run; this version measured 1.862-1.871 ms
across 6 normal-clock runs, 1.944 ms on one slow-clock day — the spread is
platform clock weather, not DMA). The PE stream is unchanged: 6912 matmuls x 512
cols, fp32r dendrites (argmax selection precision), fp16 W-path. Measured
PE floor at the platform GPIO clock cap (13/16 = 1.95 GHz; engages under
sustained PE load regardless of DMA traffic — probed with a half-traffic
bf16 run): ~1.82 ms.

Gate exchange: one 8-rank AllToAll per (layer, unit-tile) piece (bypass,
1 MB). SPMD uniformity: a core cannot index A2A shards by its own group, so
each sender mirrors its 4 gate chunks into both groups' shard slots (every
shard holds valid sigmoids, no NaN) and each receiver picks between the two
candidate shards with copy_predicated on a per-core constant mask input.
Piece u fires ~135 us before piece u+1 so the ~30 us collective latency
hides under remaining dendrite matmuls; at layer boundaries a
COVER_S-segment chunk of the next layer's dendrites is emitted before the
W block to cover piece 3's flight.

Tail: the last layer's final gate piece would leave the classifier waiting
~45 us on a collective fired after the last dendrite matmul. Instead the
4 unit tiles of classifier piece 3 ({3,7,11,15}) are computed REPLICATED —
each core for its own 512-row batch shard, gates purely local (no A2A in
the tail). That swaps 512 sharded for 512 replicated matmuls (PE count
unchanged). The replicated piece streams its segment weights in fp16
(16.8 MB, keeps tail-phase HBM demand at ~120 GB/s); fp16 operand noise on
that quarter of the units raises rel-err 1.593e-2 -> 1.650e-2 (gate 2e-2,
measured, deterministic).
Classifier accumulates kt-split over 8 concurrent PSUM banks as each gate
piece lands.

BatchNorm stays exact full-batch via a 16 KB AllReduce of per-unit
(sum, sum_sq), overlapped with dendrite matmuls. Dendrite
argmax-|.|-gather is gather-free: running max/min over the 16 segment
outputs, then sel = where(max >= -min, max, min).
"""
import os
import sys
import types

sys.path.insert(0, "/opt/trn_rl_repo")

import numpy as np

import concourse.bass as bass
import concourse.mybir as mybir
import concourse.tile as tile
from concourse import bacc
from concourse.bass_utils import run_bass_kernel_spmd

B, D, H, S, OUT = 4096, 1024, 2048, 16, 1000
CORES = 8
BS = B // CORES            # 512 rows per core (W-path shard / A2A shard)
NQ = 4                     # unit quarters
GB = B // (CORES // NQ)    # 2048 rows per dendrite batch group
UQ = H // NQ               # 512 units per core for dendrites
UT_Q = UQ // 128           # 4 unit tiles per core for dendrites
OUTP = 1024                # classifier outputs padded to 8*128
KT_D = D // 128            # 8 k-tiles for 1024-dim contractions
KT_H = H // 128            # 16 k-tiles for 2048-dim contractions
UT_H = H // 128            # 16 unit tiles per hidden layer
UT_O = OUTP // 128         # 8 unit tiles for classifier
TAILK = [k for k in range(UT_H) if k % UT_Q == 3]   # {3,7,11,15}
BN_EPS = 1e-5

F32 = mybir.dt.float32
F32R = mybir.dt.float32r
F16 = mybir.dt.float16
AX = mybir.AxisListType
ALU = mybir.AluOpType
ACTF = mybir.ActivationFunctionType

LAST_EXEC_NS = None
_CACHE = {}
DBG_NO_COLL = bool(int(os.environ.get("DBG_NO_COLL", "0")))
COVER_S = int(os.environ.get("COVER_S", "8"))


def _install_ntff_shim():
    """Register antenv.axon_hooks so bass_utils can NTFF-profile under axon."""
    if "antenv.axon_hooks" in sys.modules:
        return
    try:
        from trn_agent_boot.trn_boot import _ntff_profile_via_ctypes

        hook = _ntff_profile_via_ctypes("/opt/axon/libaxon_pjrt.so")
        mod = types.ModuleType("antenv.axon_hooks")
        state = {"hook": hook}
        mod.set_axon_ntff_profile_hook = lambda h: state.__setitem__("hook", h)
        mod.get_axon_ntff_profile_hook = lambda: state["hook"]
        sys.modules["antenv.axon_hooks"] = mod
    except Exception:
        pass


def _build_nc():
    nc = bacc.Bacc("TRN2", target_bir_lowering=False, debug=False,
                   num_devices=CORES)

    xinT = nc.dram_tensor("xinT", [KT_D, 128, BS], F16, kind="ExternalInput").ap()
    ctxT = nc.dram_tensor("ctxT", [KT_D, 128, GB], F32R, kind="ExternalInput").ap()
    # own-batch-shard ctx for the replicated tail piece
    ctx2T = nc.dram_tensor("ctx2T", [KT_D, 128, BS], F16,
                           kind="ExternalInput").ap()
    wr0 = nc.dram_tensor("wr0", [UT_H, 128, KT_D, 128], F16, kind="ExternalInput").ap()
    wr1 = nc.dram_tensor("wr1", [UT_H, 128, KT_H, 128], F16, kind="ExternalInput").ap()
    wr2 = nc.dram_tensor("wr2", [UT_H, 128, KT_H, 128], F16, kind="ExternalInput").ap()
    # classifier weights, piece-major: [out_ut, piece u, 128ki, 4r*128u]
    wcr = nc.dram_tensor("wcr", [UT_O, UT_Q, 128, NQ * 128], F16,
                         kind="ExternalInput").ap()
    # sharded segment weights (unit quarter, layers 0-2; layer 2 only ut 0..2)
    swr = [
        nc.dram_tensor(f"swr{i}", [UT_Q, S, 128, KT_D, 128], F32R,
                       kind="ExternalInput").ap()
        for i in range(3)
    ]
    # replicated fp16 tail piece: layer-2 sw for global unit tiles {3,7,11,15}
    swtr = nc.dram_tensor("swtr", [NQ, S, 128, KT_D, 128], F16,
                          kind="ExternalInput").ap()
    br = nc.dram_tensor("br", [128, 3 * UT_H], F32, kind="ExternalInput").ap()
    bcr = nc.dram_tensor("bcr", [128, UT_O], F32, kind="ExternalInput").ap()
    gmr = nc.dram_tensor("gmr", [128, BS], mybir.dt.uint8,
                         kind="ExternalInput").ap()
    outT = nc.dram_tensor("outT", [UT_O, 128, BS], F32, kind="ExternalOutput").ap()

    wr = [wr0, wr1, wr2]

    with tile.TileContext(nc) as tc:
        with (
            tc.tile_pool(name="pers", bufs=1) as pers,
            tc.tile_pool(name="wblk", bufs=3) as wpool,
            tc.tile_pool(name="wcc", bufs=8) as wcpool,
            tc.tile_pool(name="swp", bufs=6) as swpool,
            tc.tile_pool(name="mxp", bufs=2) as mxpool,
            tc.tile_pool(name="work", bufs=1) as work,
            tc.tile_pool(name="gio", bufs=2) as gio,
            tc.tile_pool(name="ob", bufs=2) as opool,
            tc.tile_pool(name="pb", bufs=2, space="PSUM") as pb,
            tc.tile_pool(name="pd", bufs=6, space="PSUM") as pd,
            tc.tile_pool(name="dram", bufs=1, space="DRAM") as dram,
        ):
            # ---- persistent tiles ----
            # xf slots first hold xin (fp16, W0 moving operand), later the
            # replicated tail piece's own-shard ctx (f32r) — same tags, the
            # pool ring (bufs=1) reuses the slot with automatic deps.
            xin_tiles = [pers.tile([128, BS], F16, tag=f"xf{k}", name=f"xin{k}")
                         for k in range(KT_D)]
            ctx_tiles = [pers.tile([128, GB], F32R, tag=f"ctx{k}", name=f"ctx{k}")
                         for k in range(KT_D)]
            h_tiles = [pers.tile([128, BS], F16, tag=f"h{k}", name=f"h{k}")
                       for k in range(UT_H)]
            y_tiles = [pers.tile([128, BS], F16, tag=f"y{k}", name=f"y{k}")
                       for k in range(UT_H)]
            bias_sb = pers.tile([128, 3 * UT_H], F32, tag="bias_sb", name="bias_sb")
            bc_sb = pers.tile([128, UT_O], F32, tag="bc_sb", name="bc_sb")
            gmask = pers.tile([128, BS], mybir.dt.uint8, tag="gmask",
                              name="gmask")

            nc.sync.dma_start(bias_sb[:], br)
            for k in range(KT_D):
                nc.sync.dma_start(xin_tiles[k][:], xinT[k])

            gin = [[dram.tile([CORES, 128, BS], F16, tag=f"gi{l}_{u}",
                              name=f"gi{l}_{u}") for u in range(UT_Q)]
                   for l in range(3)]
            gout = [[dram.tile([CORES, 128, BS], F16, tag=f"go{l}_{u}",
                               name=f"go{l}_{u}") for u in range(UT_Q)]
                    for l in range(3)]

            bn = {}  # layer -> (scale, nbias)

            def emit_w_block(layer, interleave=None):
                kt_in = KT_D if layer == 0 else KT_H
                in_tiles = xin_tiles if layer == 0 else h_tiles
                stats_loc = pers.tile([128, 2 * UT_H], F32, tag=f"stl{layer}",
                                      name=f"stl{layer}")
                for ut in range(UT_H):
                    if interleave and ut in interleave:
                        interleave[ut]()
                    wchunk = wpool.tile([128, kt_in * 128], F16, tag="wblk",
                                        name=f"w{layer}_{ut}")
                    src = wr[layer][ut].rearrange("p a b -> p (a b)")
                    if layer == 0 and ut < 2:
                        # halves so the first matmuls start after 128 KB
                        nc.sync.dma_start(wchunk[:, :kt_in * 64], src[:, :kt_in * 64])
                        nc.sync.dma_start(wchunk[:, kt_in * 64:], src[:, kt_in * 64:])
                    else:
                        nc.sync.dma_start(wchunk[:], src)
                    ps = pb.tile([128, BS], F32, tag="yblk", name=f"yp{layer}_{ut}")
                    for kt in range(kt_in):
                        nc.tensor.matmul(
                            ps[:],
                            wchunk[:, kt * 128:(kt + 1) * 128],
                            in_tiles[kt][:],
                            start=(kt == 0),
                            stop=(kt == kt_in - 1),
                        )
                    y = y_tiles[ut]
                    nc.scalar.activation(
                        y[:], ps[:], ACTF.Identity,
                        bias=bias_sb[:, layer * UT_H + ut:layer * UT_H + ut + 1],
                    )
                    nc.vector.tensor_reduce(
                        stats_loc[:, ut:ut + 1], y[:], axis=AX.X, op=ALU.add)
                    sq = work.tile([128, BS], F32, tag="sq", name=f"sq{layer}_{ut}")
                    nc.scalar.activation(
                        sq[:], y[:], ACTF.Square,
                        accum_out=stats_loc[:, UT_H + ut:UT_H + ut + 1],
                    )
                stats_glob = pers.tile([128, 2 * UT_H], F32, tag=f"stg{layer}",
                                       name=f"stg{layer}")
                if DBG_NO_COLL:
                    nc.vector.tensor_scalar_mul(stats_glob[:], stats_loc[:],
                                                float(CORES))
                else:
                    bnc_in = dram.tile([128, 2 * UT_H], F32, tag=f"bin{layer}",
                                       name=f"bin{layer}")
                    bnc_out = dram.tile([128, 2 * UT_H], F32, addr_space="Shared",
                                        tag=f"bout{layer}", name=f"bout{layer}")
                    nc.sync.dma_start(bnc_in[:], stats_loc[:])
                    nc.gpsimd.collective_compute(
                        "AllReduce", ALU.add,
                        ins=[bnc_in.opt()],
                        outs=[bnc_out.opt()],
                        replica_groups=[list(range(CORES))],
                    )
                    nc.sync.dma_start(stats_glob[:], bnc_out[:])
                return stats_glob

            def emit_bn_coeffs(layer, stats_glob):
                mean = pers.tile([128, UT_H], F32, tag=f"mean{layer}",
                                 name=f"mean{layer}")
                var = pers.tile([128, UT_H], F32, tag=f"var{layer}",
                                name=f"var{layer}")
                scale = pers.tile([128, UT_H], F32, tag=f"scale{layer}",
                                  name=f"scale{layer}")
                nbias = pers.tile([128, UT_H], F32, tag=f"nbias{layer}",
                                  name=f"nbias{layer}")
                msq = pers.tile([128, UT_H], F32, tag=f"msq{layer}",
                                name=f"msq{layer}")
                nc.vector.tensor_scalar_mul(mean[:], stats_glob[:, 0:UT_H], 1.0 / B)
                nc.vector.tensor_scalar_mul(var[:], stats_glob[:, UT_H:2 * UT_H],
                                            1.0 / B)
                nc.vector.scalar_tensor_tensor(
                    out=msq[:], in0=mean[:], scalar=-1.0, in1=mean[:],
                    op0=ALU.mult, op1=ALU.mult,
                )
                nc.vector.tensor_tensor(var[:], var[:], msq[:], op=ALU.add)
                nc.vector.tensor_scalar_add(var[:], var[:], BN_EPS)
                nc.scalar.sqrt(scale[:], var[:])
                nc.vector.reciprocal(scale[:], scale[:])
                nc.vector.scalar_tensor_tensor(
                    out=nbias[:], in0=mean[:], scalar=-1.0, in1=scale[:],
                    op0=ALU.mult, op1=ALU.mult,
                )
                bn[layer] = (scale, nbias)

            def alloc_mxmn(nm):
                mx = [mxpool.tile([128, BS], F32, tag=f"mx{bc}",
                                  name=f"mx_{nm}_{bc}") for bc in range(4)]
                mn = [mxpool.tile([128, BS], F32, tag=f"mn{bc}",
                                  name=f"mn_{nm}_{bc}") for bc in range(4)]
                return mx, mn

            def emit_dend_piece(layer, ut, s_range, mx, mn, pre=None):
                for s in s_range:
                    if pre is not None and s < len(pre):
                        swc = pre[s]
                    else:
                        swc = swpool.tile([128, KT_D * 128], F32R, tag="sw",
                                          name=f"sw{layer}_{ut}_{s}")
                        nc.sync.dma_start(
                            swc[:],
                            swr[layer][ut, s].rearrange("p a b -> p (a b)"),
                        )
                    for bc in range(4):
                        psd = pd.tile([128, BS], F32, tag="pd",
                                      name=f"pd{layer}_{ut}_{s}_{bc}")
                        for kt in range(KT_D):
                            nc.tensor.matmul(
                                psd[:],
                                swc[:, kt * 128:(kt + 1) * 128],
                                ctx_tiles[kt][:, bc * BS:(bc + 1) * BS],
                                start=(kt == 0),
                                stop=(kt == KT_D - 1),
                            )
                        if s == 0:
                            nc.scalar.copy(mx[bc][:], psd[:])
                            nc.vector.tensor_copy(mn[bc][:], psd[:])
                        else:
                            nc.vector.tensor_tensor(mx[bc][:], mx[bc][:], psd[:],
                                                    op=ALU.max)
                            nc.vector.tensor_tensor(mn[bc][:], mn[bc][:], psd[:],
                                                    op=ALU.min)

            def emit_dend_gates(layer, ut, mx, mn):
                """sel = where(mx >= -mn, mx, mn); sigmoid; mirrored shard
                writes; fire the A2A piece."""
                for bc in range(4):
                    negmn = work.tile([128, BS], F32, tag="negmn",
                                      name=f"ng{layer}_{ut}_{bc}")
                    nc.scalar.mul(negmn[:], mn[bc][:], -1.0)
                    mask = work.tile([128, BS], mybir.dt.uint8, tag="mask",
                                     name=f"mk{layer}_{ut}_{bc}")
                    nc.vector.tensor_tensor(mask[:], mx[bc][:], negmn[:],
                                            op=ALU.is_ge)
                    nc.vector.copy_predicated(mn[bc][:], mask[:], mx[bc][:])
                    gs = gio.tile([128, BS], F16, tag="gs",
                                  name=f"gs{layer}_{ut}_{bc}")
                    nc.scalar.activation(gs[:], mn[bc][:], ACTF.Sigmoid)
                    nc.sync.dma_start(gin[layer][ut][bc], gs[:])
                    nc.sync.dma_start(gin[layer][ut][bc + 4], gs[:])
                if not DBG_NO_COLL:
                    nc.gpsimd.collective_compute(
                        "AllToAll", ALU.bypass,
                        ins=[gin[layer][ut].opt()],
                        outs=[gout[layer][ut].opt()],
                        replica_groups=[list(range(CORES))],
                    )

            def emit_h_tiles(layer, ks):
                """h[k] = relu(bn(y[k])) * select(ga, gb)."""
                scale, nbias = bn[layer]
                for k in ks:
                    u = k % UT_Q
                    r = k // UT_Q
                    src = gout[layer][u]
                    if DBG_NO_COLL:
                        src = gin[layer][u]
                    ga = gio.tile([128, BS], F16, tag="ga", name=f"ga{layer}_{k}")
                    gb = gio.tile([128, BS], F16, tag="gb", name=f"gb{layer}_{k}")
                    nc.sync.dma_start(ga[:], src[r])
                    nc.sync.dma_start(gb[:], src[r + 4])
                    # group-0 cores keep ga (shard r); group-1 take gb (r+4)
                    nc.vector.copy_predicated(ga[:], gmask[:], gb[:])
                    nc.scalar.activation(
                        h_tiles[k][:], y_tiles[k][:], ACTF.Relu,
                        bias=nbias[:, k:k + 1], scale=scale[:, k:k + 1],
                    )
                    nc.vector.tensor_tensor(h_tiles[k][:], h_tiles[k][:], ga[:],
                                            op=ALU.mult)

            def emit_tail_piece():
                """Layer-2 dendrites for global unit tiles {3,7,11,15},
                replicated per core over its own 512-row batch shard (fp16
                sw), gates local — no collective in the tail."""
                scale, nbias = bn[2]
                ctx2 = _ctx2
                for t in range(NQ):
                    k = TAILK[t]
                    mx = mxpool.tile([128, BS], F32, tag=f"mx{t}", name=f"mxT{t}")
                    mn = mxpool.tile([128, BS], F32, tag=f"mn{t}", name=f"mnT{t}")
                    for s in range(S):
                        swc = swpool.tile([128, KT_D * 128], F16, tag="sw",
                                          name=f"swt{t}_{s}")
                        nc.sync.dma_start(
                            swc[:],
                            swtr[t, s].rearrange("p a b -> p (a b)"),
                        )
                        psd = pd.tile([128, BS], F32, tag="pd",
                                      name=f"pdt{t}_{s}")
                        for kt in range(KT_D):
                            nc.tensor.matmul(
                                psd[:],
                                swc[:, kt * 128:(kt + 1) * 128],
                                ctx2[kt][:],
                                start=(kt == 0),
                                stop=(kt == KT_D - 1),
                            )
                        if s == 0:
                            nc.scalar.copy(mx[:], psd[:])
                            nc.vector.tensor_copy(mn[:], psd[:])
                        else:
                            nc.vector.tensor_tensor(mx[:], mx[:], psd[:],
                                                    op=ALU.max)
                            nc.vector.tensor_tensor(mn[:], mn[:], psd[:],
                                                    op=ALU.min)
                    negmn = work.tile([128, BS], F32, tag="negmn", name=f"ngT{t}")
                    nc.scalar.mul(negmn[:], mn[:], -1.0)
                    mask = work.tile([128, BS], mybir.dt.uint8, tag="mask",
                                     name=f"mkT{t}")
                    nc.vector.tensor_tensor(mask[:], mx[:], negmn[:], op=ALU.is_ge)
                    nc.vector.copy_predicated(mn[:], mask[:], mx[:])
                    gs = gio.tile([128, BS], F16, tag="gs", name=f"gsT{t}")
                    nc.scalar.activation(gs[:], mn[:], ACTF.Sigmoid)
                    nc.scalar.activation(
                        h_tiles[k][:], y_tiles[k][:], ACTF.Relu,
                        bias=nbias[:, k:k + 1], scale=scale[:, k:k + 1],
                    )
                    nc.vector.tensor_tensor(h_tiles[k][:], h_tiles[k][:], gs[:],
                                            op=ALU.mult)

            # ================= program =================
            _ctx2 = [pers.tile([128, BS], F16, tag=f"xf{k}", name=f"c2_{k}")
                     for k in range(KT_D)]
            pre_sw = [swpool.tile([128, KT_D * 128], F32R, tag="sw",
                                  name=f"swpre{s}") for s in range(2)]

            def _issue_pre_sw():
                for s2 in range(2):
                    nc.sync.dma_start(
                        pre_sw[s2][:],
                        swr[0][0, s2].rearrange("p a b -> p (a b)"))

            def _issue_ctx():
                # quarter-major: D0's s=0 sweeps batch chunks in order, and
                # chunk bc needs columns [bc*512,(bc+1)*512) of ALL 8 kt —
                # issuing quarter-by-quarter across kt makes the first matmul
                # group's 2.1 MB arrive first instead of last
                for qtr in range(4):
                    a, b = qtr * (GB // 4), (qtr + 1) * (GB // 4)
                    for k in range(KT_D):
                        nc.sync.dma_start(ctx_tiles[k][:, a:b], ctxT[k, :, a:b])

            def _issue_late_small():
                nc.sync.dma_start(bc_sb[:], bcr)
                nc.sync.dma_start(gmask[:], gmr)

            # PE clock warmup: the HAM/GPIO un-throttle needs ~3.4 us of
            # sustained PE busy; the first real matmul waits ~17 us for DMA.
            # Dummy matmuls on zeroed scratch (never read) bridge the gap so
            # W0 starts at full clock instead of 1.2 GHz.
            warm_s = pers.tile([128, 128], F16, tag="warm_s", name="warm_s")
            warm_m = pers.tile([128, BS], F16, tag="warm_m", name="warm_m")
            nc.vector.memset(warm_s[:], 0.0)
            nc.vector.memset(warm_m[:], 0.0)
            # length tuned to end just before the earliest DMA-ready time
            # (~16 us): the residual <3.4 us idle keeps HAM warm without the
            # dummy chain ever delaying the first real matmul.
            wps = pb.tile([128, BS], F32, tag="yblk", name="warmps")
            for _ in range(100):
                nc.tensor.matmul(wps[:, :128], warm_s[:], warm_m[:, :128],
                                 start=True, stop=True)

            stats = {0: emit_w_block(0, interleave={2: _issue_pre_sw,
                                                    4: _issue_ctx,
                                                    7: _issue_late_small})}

            carry = None  # (mx, mn) of the next layer's piece-0 cover chunk
            for layer in range(3):
                n_shard = UT_Q if layer < 2 else UT_Q - 1
                for ut in range(n_shard):
                    if ut == 0 and carry is not None:
                        mx, mn = carry
                        emit_dend_piece(layer, 0, range(COVER_S, S), mx, mn)
                        carry = None
                    else:
                        mx, mn = alloc_mxmn(f"{layer}_{ut}")
                        emit_dend_piece(layer, ut, range(S), mx, mn,
                                        pre=pre_sw if (layer, ut) == (0, 0)
                                        else None)
                    emit_dend_gates(layer, ut, mx, mn)
                    if ut == 0:
                        emit_bn_coeffs(layer, stats[layer])

                if layer < 2:
                    ks_early = [k for k in range(UT_H) if k % UT_Q < 3]
                    ks_late = [k for k in range(UT_H) if k % UT_Q == 3]
                    emit_h_tiles(layer, ks_early)
                    cmx, cmn = alloc_mxmn(f"c{layer + 1}")
                    emit_dend_piece(layer + 1, 0, range(COVER_S), cmx, cmn)
                    emit_h_tiles(layer, ks_late)
                    stats[layer + 1] = emit_w_block(layer + 1)
                    carry = (cmx, cmn)
                    if layer == 0:
                        # tail-piece ctx: xf slots are free once W0 read xin;
                        # emitting here lets the DMA run ~1 ms early
                        for k in range(KT_D):
                            nc.sync.dma_start(_ctx2[k][:], ctx2T[k])

            # layer-2 replicated tail piece (gates local, h for {3,7,11,15})
            emit_tail_piece()

            # ---- classifier (kt-split accumulation over 8 PSUM banks) ----
            cps = [(pd if i < 6 else pb).tile([128, BS], F32,
                                              tag=("pd" if i < 6 else "yblk"),
                                              name=f"cps{i}")
                   for i in range(UT_O)]
            for u in range(UT_Q):
                ks = [r * UT_Q + u for r in range(NQ)]
                if u < 3:
                    emit_h_tiles(2, ks)   # u==3 h tiles came from the tail piece
                for o in range(UT_O):
                    wcc = wcpool.tile([128, NQ * 128], F16, tag="wcc",
                                      name=f"wcc{u}_{o}")
                    nc.sync.dma_start(wcc[:], wcr[o, u])
                    for j, k in enumerate(ks):
                        nc.tensor.matmul(
                            cps[o][:],
                            wcc[:, j * 128:(j + 1) * 128],
                            h_tiles[k][:],
                            start=(u == 0 and j == 0),
                            stop=(u == UT_Q - 1 and j == NQ - 1),
                        )
                    if u == UT_Q - 1:
                        osb = opool.tile([128, BS], F32, tag="osb",
                                         name=f"osb{o}")
                        nc.scalar.activation(osb[:], cps[o][:], ACTF.Identity,
                                             bias=bc_sb[:, o:o + 1])
                        nc.sync.dma_start(outT[o], osb[:])

    nc.compile()
    return nc


def _prep_host(x, w0, b0, sw0, w1, b1, sw1, w2, b2, sw2, wc, bc):
    f = np.float32
    h16 = np.float16

    def _w_reorder(w, kt):  # w [H_out, K] -> [ut, 128ki, kt, 128u]
        wT = np.ascontiguousarray(w.astype(h16).T)        # [K, H_out]
        K, HO = wT.shape
        return np.ascontiguousarray(
            wT.reshape(kt, 128, HO // 128, 128).transpose(2, 1, 0, 3))

    wc_pad = np.zeros((OUTP, H), f)
    wc_pad[:OUT] = wc.astype(f)
    bc_pad = np.zeros((OUTP,), f)
    bc_pad[:OUT] = bc.astype(f)

    # classifier: [out_ut, 128ki, kt=16, 128u], kt = 4r+u
    #   -> [out_ut, piece u, 128ki, (r, 128u)]
    wcr = _w_reorder(wc_pad, KT_H)                        # [8, 128, 16, 128]
    wcr = np.ascontiguousarray(
        wcr.reshape(UT_O, 128, NQ, UT_Q, 128).transpose(0, 3, 1, 2, 4)
        .reshape(UT_O, UT_Q, 128, NQ * 128))

    def _sw_reorder(sl, dt):  # [U, S, D] -> [U/128, S, 128ki, kt, 128u]
        nt = sl.shape[0] // 128
        return np.ascontiguousarray(
            sl.astype(dt).reshape(nt, 128, S, KT_D, 128).transpose(0, 2, 4, 3, 1))

    # replicated fp16 tail piece: layer-2 tiles {3,7,11,15}
    swt = np.concatenate([sw2[k * 128:(k + 1) * 128] for k in TAILK], axis=0)

    sws = [sw0, sw1, sw2]
    common = {
        "wr0": _w_reorder(w0, KT_D),
        "wr1": _w_reorder(w1, KT_H),
        "wr2": _w_reorder(w2, KT_H),
        "wcr": wcr,
        "swtr": _sw_reorder(swt, h16),
        "br": np.ascontiguousarray(
            np.stack([b0, b1, b2]).astype(f).reshape(3 * UT_H, 128).T),
        "bcr": np.ascontiguousarray(bc_pad.reshape(UT_O, 128).T),
    }
    in_maps = []
    for c in range(CORES):
        q, r = c // NQ, c % NQ
        xs = x[c * BS:(c + 1) * BS]
        xg = x[q * GB:(q + 1) * GB]
        m = dict(common)
        m["xinT"] = np.ascontiguousarray(
            xs[:, :D].astype(h16).T).reshape(KT_D, 128, BS)
        m["ctxT"] = np.ascontiguousarray(
            xg[:, D:].astype(f).T).reshape(KT_D, 128, GB)
        m["ctx2T"] = np.ascontiguousarray(
            xs[:, D:].astype(h16).T).reshape(KT_D, 128, BS)
        for i, sw in enumerate(sws):
            m[f"swr{i}"] = _sw_reorder(sw[r * UQ:(r + 1) * UQ], f)
        m["gmr"] = np.full((128, BS), q, np.uint8)
        in_maps.append(m)
    return in_maps


def kernel(**inputs):
    global LAST_EXEC_NS
    if "nc" not in _CACHE:
        _CACHE["nc"] = _build_nc()
    nc = _CACHE["nc"]

    in_maps = _prep_host(**inputs)

    trace = bool(int(os.environ.get("KERNEL_TRACE", "0")))
    if trace:
        _install_ntff_shim()

    tdir = None
    if trace:
        tdir = os.environ.get("KERNEL_TRACE_DIR")
        if tdir:
            os.makedirs(tdir, exist_ok=True)
    res = run_bass_kernel_spmd(nc, in_maps, core_ids=list(range(CORES)),
                               trace=trace, tmpdir=tdir)
    LAST_EXEC_NS = res.exec_time_ns

    out = np.empty((B, OUT), np.float32)
    for c in range(CORES):
        oT = res.results[c]["outT"].reshape(OUTP, BS)
        out[c * BS:(c + 1) * BS] = oT[:OUT].T
    return out


# revision 20
# speedup vs baseline: 1.0474x; 1.0474x over previous
"""DendriticMLP Trainium2 kernel — 8-core hybrid data/tensor parallel.

v3 vs baseline (pure data-parallel): the dendrite path (89% of matmul
cycles, 402 MB/core of fp32r segment-weight streaming when replicated) is
tensor-sharded: core c computes d = ctx @ sw for unit quarter r = c % 4
(512 units) over batch group q = c // 4 (2048 rows). Segment-weight traffic
drops to ~120 MB/core (~170 MB total vs 433 MB), removing the HBM
co-binding that made replicated runs stochastically DMA-bound (baseline
measured 1.85-2.10 ms run-to-run; this version measured 1.862-1.871 ms
across 6 normal-clock runs, 1.944 ms on one slow-clock day — the spread is
platform clock weather, not DMA). The PE stream is unchanged: 6912 matmuls x 512
cols, fp32r dendrites (argmax selection precision), fp16 W-path. Measured
PE floor at the platform GPIO clock cap (13/16 = 1.95 GHz; engages under
sustained PE load regardless of DMA traffic — probed with a half-traffic
bf16 run): ~1.82 ms.

Gate exchange: one 8-rank AllToAll per (layer, unit-tile) piece (bypass,
1 MB). SPMD uniformity: a core cannot index A2A shards by its own group, so
each sender mirrors its 4 gate chunks into both groups' shard slots (every
shard holds valid sigmoids, no NaN) and each receiver picks between the two
candidate shards with copy_predicated on a per-core constant mask input.
Piece u fires ~135 us before piece u+1 so the ~30 us collective latency
hides under remaining dendrite matmuls; at layer boundaries a
COVER_S-segment chunk of the next layer's dendrites is emitted before the
W block to cover piece 3's flight.

Tail: the last layer's final gate piece would leave the classifier waiting
~45 us on a collective fired after the last dendrite matmul. Instead the
4 unit tiles of classifier piece 3 ({3,7,11,15}) are computed REPLICATED —
each core for its own 512-row batch shard, gates purely local (no A2A in
the tail). That swaps 512 sharded for 512 replicated matmuls (PE count
unchanged). The replicated piece streams its segment weights in fp16
(16.8 MB, keeps tail-phase HBM demand at ~120 GB/s); fp16 operand noise on
that quarter of the units raises rel-err 1.593e-2 -> 1.650e-2 (gate 2e-2,
measured, deterministic).
Classifier accumulates kt-split over 8 concurrent PSUM banks as each gate
piece lands.

BatchNorm stays exact full-batch via a 16 KB AllReduce of per-unit
(sum, sum_sq), overlapped with dendrite matmuls. Dendrite
argmax-|.|-gather is gather-free: running max/min over the 16 segment
outputs, then sel = where(max >= -min, max, min).
"""
import os
import sys
import types

sys.path.insert(0, "/opt/trn_rl_repo")

import numpy as np

import concourse.bass as bass
import concourse.mybir as mybir
import concourse.tile as tile
from concourse import bacc
from concourse.bass_utils import run_bass_kernel_spmd

B, D, H, S, OUT = 4096, 1024, 2048, 16, 1000
CORES = 8
BS = B // CORES            # 512 rows per core (W-path shard / A2A shard)
NQ = 4                     # unit quarters
GB = B // (CORES // NQ)    # 2048 rows per dendrite batch group
UQ = H // NQ               # 512 units per core for dendrites
UT_Q = UQ // 128           # 4 unit tiles per core for dendrites
OUTP = 1024                # classifier outputs padded to 8*128
KT_D = D // 128            # 8 k-tiles for 1024-dim contractions
KT_H = H // 128            # 16 k-tiles for 2048-dim contractions
UT_H = H // 128            # 16 unit tiles per hidden layer
UT_O = OUTP // 128         # 8 unit tiles for classifier
TAILK = [k for k in range(UT_H) if k % UT_Q == 3]   # {3,7,11,15}
BN_EPS = 1e-5

F32 = mybir.dt.float32
F32R = mybir.dt.float32r
F16 = mybir.dt.float16
AX = mybir.AxisListType
ALU = mybir.AluOpType
ACTF = mybir.ActivationFunctionType

LAST_EXEC_NS = None
_CACHE = {}
DBG_NO_COLL = bool(int(os.environ.get("DBG_NO_COLL", "0")))
COVER_S = int(os.environ.get("COVER_S", "8"))


def _install_ntff_shim():
    """Register antenv.axon_hooks so bass_utils can NTFF-profile under axon."""
    if "antenv.axon_hooks" in sys.modules:
        return
    try:
        from trn_agent_boot.trn_boot import _ntff_profile_via_ctypes

        hook = _ntff_profile_via_ctypes("/opt/axon/libaxon_pjrt.so")
        mod = types.ModuleType("antenv.axon_hooks")
        state = {"hook": hook}
        mod.set_axon_ntff_profile_hook = lambda h: state.__setitem__("hook", h)
        mod.get_axon_ntff_profile_hook = lambda: state["hook"]
        sys.modules["antenv.axon_hooks"] = mod
    except Exception:
        pass


def _build_nc():
    nc = bacc.Bacc("TRN2", target_bir_lowering=False, debug=False,
                   num_devices=CORES)

    xinT = nc.dram_tensor("xinT", [KT_D, 128, BS], F16, kind="ExternalInput").ap()
    ctxT = nc.dram_tensor("ctxT", [KT_D, 128, GB], F32R, kind="ExternalInput").ap()
    # own-batch-shard ctx for the replicated tail piece
    ctx2T = nc.dram_tensor("ctx2T", [KT_D, 128, BS], F16,
                           kind="ExternalInput").ap()
    wr0 = nc.dram_tensor("wr0", [UT_H, 128, KT_D, 128], F16, kind="ExternalInput").ap()
    wr1 = nc.dram_tensor("wr1", [UT_H, 128, KT_H, 128], F16, kind="ExternalInput").ap()
    wr2 = nc.dram_tensor("wr2", [UT_H, 128, KT_H, 128], F16, kind="ExternalInput").ap()
    # classifier weights, piece-major: [out_ut, piece u, 128ki, 4r*128u]
    wcr = nc.dram_tensor("wcr", [UT_O, UT_Q, 128, NQ * 128], F16,
                         kind="ExternalInput").ap()
    # sharded segment weights (unit quarter, layers 0-2; layer 2 only ut 0..2)
    swr = [
        nc.dram_tensor(f"swr{i}", [UT_Q, S, 128, KT_D, 128], F32R,
                       kind="ExternalInput").ap()
        for i in range(3)
    ]
    # replicated fp16 tail piece: layer-2 sw for global unit tiles {3,7,11,15}
    swtr = nc.dram_tensor("swtr", [NQ, S, 128, KT_D, 128], F16,
                          kind="ExternalInput").ap()
    br = nc.dram_tensor("br", [128, 3 * UT_H], F32, kind="ExternalInput").ap()
    bcr = nc.dram_tensor("bcr", [128, UT_O], F32, kind="ExternalInput").ap()
    gmr = nc.dram_tensor("gmr", [128, BS], mybir.dt.uint8,
                         kind="ExternalInput").ap()
    outT = nc.dram_tensor("outT", [UT_O, 128, BS], F32, kind="ExternalOutput").ap()

    wr = [wr0, wr1, wr2]

    with tile.TileContext(nc) as tc:
        with (
            tc.tile_pool(name="pers", bufs=1) as pers,
            tc.tile_pool(name="wblk", bufs=3) as wpool,
            tc.tile_pool(name="wcc", bufs=8) as wcpool,
            tc.tile_pool(name="swp", bufs=6) as swpool,
            tc.tile_pool(name="mxp", bufs=2) as mxpool,
            tc.tile_pool(name="work", bufs=1) as work,
            tc.tile_pool(name="gio", bufs=2) as gio,
            tc.tile_pool(name="ob", bufs=2) as opool,
            tc.tile_pool(name="pb", bufs=2, space="PSUM") as pb,
            tc.tile_pool(name="pd", bufs=6, space="PSUM") as pd,
            tc.tile_pool(name="dram", bufs=1, space="DRAM") as dram,
        ):
            # ---- persistent tiles ----
            # xf slots first hold xin (fp16, W0 moving operand), later the
            # replicated tail piece's own-shard ctx (f32r) — same tags, the
            # pool ring (bufs=1) reuses the slot with automatic deps.
            xin_tiles = [pers.tile([128, BS], F16, tag=f"xf{k}", name=f"xin{k}")
                         for k in range(KT_D)]
            ctx_tiles = [pers.tile([128, GB], F32R, tag=f"ctx{k}", name=f"ctx{k}")
                         for k in range(KT_D)]
            h_tiles = [pers.tile([128, BS], F16, tag=f"h{k}", name=f"h{k}")
                       for k in range(UT_H)]
            y_tiles = [pers.tile([128, BS], F16, tag=f"y{k}", name=f"y{k}")
                       for k in range(UT_H)]
            bias_sb = pers.tile([128, 3 * UT_H], F32, tag="bias_sb", name="bias_sb")
            bc_sb = pers.tile([128, UT_O], F32, tag="bc_sb", name="bc_sb")
            gmask = pers.tile([128, BS], mybir.dt.uint8, tag="gmask",
                              name="gmask")

            nc.sync.dma_start(bias_sb[:], br)
            for k in range(KT_D):
                nc.sync.dma_start(xin_tiles[k][:], xinT[k])

            gin = [[dram.tile([CORES, 128, BS], F16, tag=f"gi{l}_{u}",
                              name=f"gi{l}_{u}") for u in range(UT_Q)]
                   for l in range(3)]
            gout = [[dram.tile([CORES, 128, BS], F16, tag=f"go{l}_{u}",
                               name=f"go{l}_{u}") for u in range(UT_Q)]
                    for l in range(3)]

            bn = {}  # layer -> (scale, nbias)

            def emit_w_block(layer, interleave=None):
                kt_in = KT_D if layer == 0 else KT_H
                in_tiles = xin_tiles if layer == 0 else h_tiles
                stats_loc = pers.tile([128, 2 * UT_H], F32, tag=f"stl{layer}",
                                      name=f"stl{layer}")
                for ut in range(UT_H):
                    if interleave and ut in interleave:
                        interleave[ut]()
                    wchunk = wpool.tile([128, kt_in * 128], F16, tag="wblk",
                                        name=f"w{layer}_{ut}")
                    src = wr[layer][ut].rearrange("p a b -> p (a b)")
                    if layer == 0 and ut < 2:
                        # halves so the first matmuls start after 128 KB
                        nc.sync.dma_start(wchunk[:, :kt_in * 64], src[:, :kt_in * 64])
                        nc.sync.dma_start(wchunk[:, kt_in * 64:], src[:, kt_in * 64:])
                    else:
                        nc.sync.dma_start(wchunk[:], src)
                    ps = pb.tile([128, BS], F32, tag="yblk", name=f"yp{layer}_{ut}")
                    for kt in range(kt_in):
                        nc.tensor.matmul(
                            ps[:],
                            wchunk[:, kt * 128:(kt + 1) * 128],
                            in_tiles[kt][:],
                            start=(kt == 0),
                            stop=(kt == kt_in - 1),
                        )
                    y = y_tiles[ut]
                    nc.scalar.activation(
                        y[:], ps[:], ACTF.Identity,
                        bias=bias_sb[:, layer * UT_H + ut:layer * UT_H + ut + 1],
                    )
                    nc.vector.tensor_reduce(
                        stats_loc[:, ut:ut + 1], y[:], axis=AX.X, op=ALU.add)
                    sq = work.tile([128, BS], F32, tag="sq", name=f"sq{layer}_{ut}")
                    nc.scalar.activation(
                        sq[:], y[:], ACTF.Square,
                        accum_out=stats_loc[:, UT_H + ut:UT_H + ut + 1],
                    )
                stats_glob = pers.tile([128, 2 * UT_H], F32, tag=f"stg{layer}",
                                       name=f"stg{layer}")
                if DBG_NO_COLL:
                    nc.vector.tensor_scalar_mul(stats_glob[:], stats_loc[:],
                                                float(CORES))
                else:
                    bnc_in = dram.tile([128, 2 * UT_H], F32, tag=f"bin{layer}",
                                       name=f"bin{layer}")
                    bnc_out = dram.tile([128, 2 * UT_H], F32, addr_space="Shared",
                                        tag=f"bout{layer}", name=f"bout{layer}")
                    nc.sync.dma_start(bnc_in[:], stats_loc[:])
                    nc.gpsimd.collective_compute(
                        "AllReduce", ALU.add,
                        ins=[bnc_in.opt()],
                        outs=[bnc_out.opt()],
                        replica_groups=[list(range(CORES))],
                    )
                    nc.sync.dma_start(stats_glob[:], bnc_out[:])
                return stats_glob

            def emit_bn_coeffs(layer, stats_glob):
                mean = pers.tile([128, UT_H], F32, tag=f"mean{layer}",
                                 name=f"mean{layer}")
                var = pers.tile([128, UT_H], F32, tag=f"var{layer}",
                                name=f"var{layer}")
                scale = pers.tile([128, UT_H], F32, tag=f"scale{layer}",
                                  name=f"scale{layer}")
                nbias = pers.tile([128, UT_H], F32, tag=f"nbias{layer}",
                                  name=f"nbias{layer}")
                msq = pers.tile([128, UT_H], F32, tag=f"msq{layer}",
                                name=f"msq{layer}")
                nc.vector.tensor_scalar_mul(mean[:], stats_glob[:, 0:UT_H], 1.0 / B)
                nc.vector.tensor_scalar_mul(var[:], stats_glob[:, UT_H:2 * UT_H],
                                            1.0 / B)
                nc.vector.scalar_tensor_tensor(
                    out=msq[:], in0=mean[:], scalar=-1.0, in1=mean[:],
                    op0=ALU.mult, op1=ALU.mult,
                )
                nc.vector.tensor_tensor(var[:], var[:], msq[:], op=ALU.add)
                nc.vector.tensor_scalar_add(var[:], var[:], BN_EPS)
                nc.scalar.sqrt(scale[:], var[:])
                nc.vector.reciprocal(scale[:], scale[:])
                nc.vector.scalar_tensor_tensor(
                    out=nbias[:], in0=mean[:], scalar=-1.0, in1=scale[:],
                    op0=ALU.mult, op1=ALU.mult,
                )
                bn[layer] = (scale, nbias)

            def alloc_mxmn(nm):
                mx = [mxpool.tile([128, BS], F32, tag=f"mx{bc}",
                                  name=f"mx_{nm}_{bc}") for bc in range(4)]
                mn = [mxpool.tile([128, BS], F32, tag=f"mn{bc}",
                                  name=f"mn_{nm}_{bc}") for bc in range(4)]
                return mx, mn

            def emit_dend_piece(layer, ut, s_range, mx, mn, pre=None):
                for s in s_range:
                    if pre is not None and s < len(pre):
                        swc = pre[s]
                    else:
                        swc = swpool.tile([128, KT_D * 128], F32R, tag="sw",
                                          name=f"sw{layer}_{ut}_{s}")
                        nc.sync.dma_start(
                            swc[:],
                            swr[layer][ut, s].rearrange("p a b -> p (a b)"),
                        )
                    for bc in range(4):
                        psd = pd.tile([128, BS], F32, tag="pd",
                                      name=f"pd{layer}_{ut}_{s}_{bc}")
                        for kt in range(KT_D):
                            nc.tensor.matmul(
                                psd[:],
                                swc[:, kt * 128:(kt + 1) * 128],
                                ctx_tiles[kt][:, bc * BS:(bc + 1) * BS],
                                start=(kt == 0),
                                stop=(kt == KT_D - 1),
                            )
                        if s == 0:
                            nc.scalar.copy(mx[bc][:], psd[:])
                            nc.vector.tensor_copy(mn[bc][:], psd[:])
                        else:
                            nc.vector.tensor_tensor(mx[bc][:], mx[bc][:], psd[:],
                                                    op=ALU.max)
                            nc.vector.tensor_tensor(mn[bc][:], mn[bc][:], psd[:],
                                                    op=ALU.min)

            def emit_dend_gates(layer, ut, mx, mn):
                """sel = where(mx >= -mn, mx, mn); sigmoid; mirrored shard
                writes; fire the A2A piece."""
                for bc in range(4):
                    negmn = work.tile([128, BS], F32, tag="negmn",
                                      name=f"ng{layer}_{ut}_{bc}")
                    nc.scalar.mul(negmn[:], mn[bc][:], -1.0)
                    mask = work.tile([128, BS], mybir.dt.uint8, tag="mask",
                                     name=f"mk{layer}_{ut}_{bc}")
                    nc.vector.tensor_tensor(mask[:], mx[bc][:], negmn[:],
                                            op=ALU.is_ge)
                    nc.vector.copy_predicated(mn[bc][:], mask[:], mx[bc][:])
                    gs = gio.tile([128, BS], F16, tag="gs",
                                  name=f"gs{layer}_{ut}_{bc}")
                    nc.scalar.activation(gs[:], mn[bc][:], ACTF.Sigmoid)
                    nc.sync.dma_start(gin[layer][ut][bc], gs[:])
                    nc.sync.dma_start(gin[layer][ut][bc + 4], gs[:])
                if not DBG_NO_COLL:
                    nc.gpsimd.collective_compute(
                        "AllToAll", ALU.bypass,
                        ins=[gin[layer][ut].opt()],
                        outs=[gout[layer][ut].opt()],
                        replica_groups=[list(range(CORES))],
                    )

            def emit_h_tiles(layer, ks):
                """h[k] = relu(bn(y[k])) * select(ga, gb)."""
                scale, nbias = bn[layer]
                for k in ks:
                    u = k % UT_Q
                    r = k // UT_Q
                    src = gout[layer][u]
                    if DBG_NO_COLL:
                        src = gin[layer][u]
                    ga = gio.tile([128, BS], F16, tag="ga", name=f"ga{layer}_{k}")
                    gb = gio.tile([128, BS], F16, tag="gb", name=f"gb{layer}_{k}")
                    nc.sync.dma_start(ga[:], src[r])
                    nc.sync.dma_start(gb[:], src[r + 4])
                    # group-0 cores keep ga (shard r); group-1 take gb (r+4)
                    nc.vector.copy_predicated(ga[:], gmask[:], gb[:])
                    nc.scalar.activation(
                        h_tiles[k][:], y_tiles[k][:], ACTF.Relu,
                        bias=nbias[:, k:k + 1], scale=scale[:, k:k + 1],
                    )
                    nc.vector.tensor_tensor(h_tiles[k][:], h_tiles[k][:], ga[:],
                                            op=ALU.mult)

            def emit_tail_piece():
                """Layer-2 dendrites for global unit tiles {3,7,11,15},
                replicated per core over its own 512-row batch shard (fp16
                sw), gates local — no collective in the tail."""
                scale, nbias = bn[2]
                ctx2 = _ctx2
                for t in range(NQ):
                    k = TAILK[t]
                    mx = mxpool.tile([128, BS], F32, tag=f"mx{t}", name=f"mxT{t}")
                    mn = mxpool.tile([128, BS], F32, tag=f"mn{t}", name=f"mnT{t}")
                    for s in range(S):
                        swc = swpool.tile([128, KT_D * 128], F16, tag="sw",
                                          name=f"swt{t}_{s}")
                        nc.sync.dma_start(
                            swc[:],
                            swtr[t, s].rearrange("p a b -> p (a b)"),
                        )
                        psd = pd.tile([128, BS], F32, tag="pd",
                                      name=f"pdt{t}_{s}")
                        for kt in range(KT_D):
                            nc.tensor.matmul(
                                psd[:],
                                swc[:, kt * 128:(kt + 1) * 128],
                                ctx2[kt][:],
                                start=(kt == 0),
                                stop=(kt == KT_D - 1),
                            )
                        if s == 0:
                            nc.scalar.copy(mx[:], psd[:])
                            nc.vector.tensor_copy(mn[:], psd[:])
                        else:
                            nc.vector.tensor_tensor(mx[:], mx[:], psd[:],
                                                    op=ALU.max)
                            nc.vector.tensor_tensor(mn[:], mn[:], psd[:],
                                                    op=ALU.min)
                    negmn = work.tile([128, BS], F32, tag="negmn", name=f"ngT{t}")
                    nc.scalar.mul(negmn[:], mn[:], -1.0)
                    mask = work.tile([128, BS], mybir.dt.uint8, tag="mask",
                                     name=f"mkT{t}")
                    nc.vector.tensor_tensor(mask[:], mx[:], negmn[:], op=ALU.is_ge)
                    nc.vector.copy_predicated(mn[:], mask[:], mx[:])
                    gs = gio.tile([128, BS], F16, tag="gs", name=f"gsT{t}")
                    nc.scalar.activation(gs[:], mn[:], ACTF.Sigmoid)
                    nc.scalar.activation(
                        h_tiles[k][:], y_tiles[k][:], ACTF.Relu,
                        bias=nbias[:, k:k + 1], scale=scale[:, k:k + 1],
                    )
                    nc.vector.tensor_tensor(h_tiles[k][:], h_tiles[k][:], gs[:],
                                            op=ALU.mult)

            # ================= program =================
            _ctx2 = [pers.tile([128, BS], F16, tag=f"xf{k}", name=f"c2_{k}")
                     for k in range(KT_D)]
            pre_sw = [swpool.tile([128, KT_D * 128], F32R, tag="sw",
                                  name=f"swpre{s}") for s in range(2)]

            def _issue_pre_sw():
                for s2 in range(2):
                    nc.sync.dma_start(
                        pre_sw[s2][:],
                        swr[0][0, s2].rearrange("p a b -> p (a b)"))

            def _issue_ctx():
                # quarter-major: D0's s=0 sweeps batch chunks in order, and
                # chunk bc needs columns [bc*512,(bc+1)*512) of ALL 8 kt —
                # issuing quarter-by-quarter across kt makes the first matmul
                # group's 2.1 MB arrive first instead of last
                for qtr in range(4):
                    a, b = qtr * (GB // 4), (qtr + 1) * (GB // 4)
                    for k in range(KT_D):
                        nc.sync.dma_start(ctx_tiles[k][:, a:b], ctxT[k, :, a:b])

            def _issue_late_small():
                nc.sync.dma_start(bc_sb[:], bcr)
                nc.sync.dma_start(gmask[:], gmr)

            # CC-channel warmup: the first collective of a NEFF costs ~68 us
            # (cold channel + peer arrival skew); pay it at t=0 under full
            # PE cover so the BN AllReduces and gate AllToAlls run warm.
            wci_sb = pers.tile([128, 4], F32, tag="wci", name="wci")
            nc.vector.memset(wci_sb[:], 0.0)
            wcc_in = dram.tile([128, 4], F32, tag="wcc_in", name="wcc_in")
            wcc_out = dram.tile([128, 4], F32, addr_space="Shared",
                                tag="wcc_out", name="wcc_out")
            nc.sync.dma_start(wcc_in[:], wci_sb[:])
            if not DBG_NO_COLL:
                nc.gpsimd.collective_compute(
                    "AllReduce", ALU.add,
                    ins=[wcc_in.opt()], outs=[wcc_out.opt()],
                    replica_groups=[list(range(CORES))],
                )

            # PE clock warmup: the HAM/GPIO un-throttle needs ~3.4 us of
            # sustained PE busy; the first real matmul waits ~17 us for DMA.
            # Dummy matmuls on zeroed scratch (never read) bridge the gap so
            # W0 starts at full clock instead of 1.2 GHz.
            warm_s = pers.tile([128, 128], F16, tag="warm_s", name="warm_s")
            warm_m = pers.tile([128, BS], F16, tag="warm_m", name="warm_m")
            nc.vector.memset(warm_s[:], 0.0)
            nc.vector.memset(warm_m[:], 0.0)
            # length tuned to end just before the earliest DMA-ready time
            # (~16 us): the residual <3.4 us idle keeps HAM warm without the
            # dummy chain ever delaying the first real matmul.
            wps = pb.tile([128, BS], F32, tag="yblk", name="warmps")
            for _ in range(100):
                nc.tensor.matmul(wps[:, :128], warm_s[:], warm_m[:, :128],
                                 start=True, stop=True)

            stats = {0: emit_w_block(0, interleave={2: _issue_pre_sw,
                                                    4: _issue_ctx,
                                                    7: _issue_late_small})}

            carry = None  # (mx, mn) of the next layer's piece-0 cover chunk
            for layer in range(3):
                n_shard = UT_Q if layer < 2 else UT_Q - 1
                for ut in range(n_shard):
                    if ut == 0 and carry is not None:
                        mx, mn = carry
                        emit_dend_piece(layer, 0, range(COVER_S, S), mx, mn)
                        carry = None
                    else:
                        mx, mn = alloc_mxmn(f"{layer}_{ut}")
                        emit_dend_piece(layer, ut, range(S), mx, mn,
                                        pre=pre_sw if (layer, ut) == (0, 0)
                                        else None)
                    emit_dend_gates(layer, ut, mx, mn)

                if layer < 2:
                    ks_early = [k for k in range(UT_H) if k % UT_Q < 3]
                    ks_late = [k for k in range(UT_H) if k % UT_Q == 3]
                    emit_bn_coeffs(layer, stats[layer])
                    emit_h_tiles(layer, ks_early)
                    cmx, cmn = alloc_mxmn(f"c{layer + 1}")
                    emit_dend_piece(layer + 1, 0, range(COVER_S), cmx, cmn)
                    emit_h_tiles(layer, ks_late)
                    stats[layer + 1] = emit_w_block(layer + 1)
                    carry = (cmx, cmn)
                    if layer == 0:
                        # tail-piece ctx: xf slots are free once W0 read xin;
                        # emitting here lets the DMA run ~1 ms early
                        for k in range(KT_D):
                            nc.sync.dma_start(_ctx2[k][:], ctx2T[k])

            # layer-2 replicated tail piece (gates local, h for {3,7,11,15})
            emit_bn_coeffs(2, stats[2])
            emit_tail_piece()

            # ---- classifier (kt-split accumulation over 8 PSUM banks) ----
            cps = [(pd if i < 6 else pb).tile([128, BS], F32,
                                              tag=("pd" if i < 6 else "yblk"),
                                              name=f"cps{i}")
                   for i in range(UT_O)]
            for u in range(UT_Q):
                ks = [r * UT_Q + u for r in range(NQ)]
                if u < 3:
                    emit_h_tiles(2, ks)   # u==3 h tiles came from the tail piece
                for o in range(UT_O):
                    wcc = wcpool.tile([128, NQ * 128], F16, tag="wcc",
                                      name=f"wcc{u}_{o}")
                    nc.sync.dma_start(wcc[:], wcr[o, u])
                    for j, k in enumerate(ks):
                        nc.tensor.matmul(
                            cps[o][:],
                            wcc[:, j * 128:(j + 1) * 128],
                            h_tiles[k][:],
                            start=(u == 0 and j == 0),
                            stop=(u == UT_Q - 1 and j == NQ - 1),
                        )
                    if u == UT_Q - 1:
                        osb = opool.tile([128, BS], F32, tag="osb",
                                         name=f"osb{o}")
                        nc.scalar.activation(osb[:], cps[o][:], ACTF.Identity,
                                             bias=bc_sb[:, o:o + 1])
                        nc.sync.dma_start(outT[o], osb[:])

    nc.compile()
    return nc


def _prep_host(x, w0, b0, sw0, w1, b1, sw1, w2, b2, sw2, wc, bc):
    f = np.float32
    h16 = np.float16

    def _w_reorder(w, kt):  # w [H_out, K] -> [ut, 128ki, kt, 128u]
        wT = np.ascontiguousarray(w.astype(h16).T)        # [K, H_out]
        K, HO = wT.shape
        return np.ascontiguousarray(
            wT.reshape(kt, 128, HO // 128, 128).transpose(2, 1, 0, 3))

    wc_pad = np.zeros((OUTP, H), f)
    wc_pad[:OUT] = wc.astype(f)
    bc_pad = np.zeros((OUTP,), f)
    bc_pad[:OUT] = bc.astype(f)

    # classifier: [out_ut, 128ki, kt=16, 128u], kt = 4r+u
    #   -> [out_ut, piece u, 128ki, (r, 128u)]
    wcr = _w_reorder(wc_pad, KT_H)                        # [8, 128, 16, 128]
    wcr = np.ascontiguousarray(
        wcr.reshape(UT_O, 128, NQ, UT_Q, 128).transpose(0, 3, 1, 2, 4)
        .reshape(UT_O, UT_Q, 128, NQ * 128))

    def _sw_reorder(sl, dt):  # [U, S, D] -> [U/128, S, 128ki, kt, 128u]
        nt = sl.shape[0] // 128
        return np.ascontiguousarray(
            sl.astype(dt).reshape(nt, 128, S, KT_D, 128).transpose(0, 2, 4, 3, 1))

    # replicated fp16 tail piece: layer-2 tiles {3,7,11,15}
    swt = np.concatenate([sw2[k * 128:(k + 1) * 128] for k in TAILK], axis=0)

    sws = [sw0, sw1, sw2]
    common = {
        "wr0": _w_reorder(w0, KT_D),
        "wr1": _w_reorder(w1, KT_H),
        "wr2": _w_reorder(w2, KT_H),
        "wcr": wcr,
        "swtr": _sw_reorder(swt, h16),
        "br": np.ascontiguousarray(
            np.stack([b0, b1, b2]).astype(f).reshape(3 * UT_H, 128).T),
        "bcr": np.ascontiguousarray(bc_pad.reshape(UT_O, 128).T),
    }
    in_maps = []
    for c in range(CORES):
        q, r = c // NQ, c % NQ
        xs = x[c * BS:(c + 1) * BS]
        xg = x[q * GB:(q + 1) * GB]
        m = dict(common)
        m["xinT"] = np.ascontiguousarray(
            xs[:, :D].astype(h16).T).reshape(KT_D, 128, BS)
        m["ctxT"] = np.ascontiguousarray(
            xg[:, D:].astype(f).T).reshape(KT_D, 128, GB)
        m["ctx2T"] = np.ascontiguousarray(
            xs[:, D:].astype(h16).T).reshape(KT_D, 128, BS)
        for i, sw in enumerate(sws):
            m[f"swr{i}"] = _sw_reorder(sw[r * UQ:(r + 1) * UQ], f)
        m["gmr"] = np.full((128, BS), q, np.uint8)
        in_maps.append(m)
    return in_maps


def kernel(**inputs):
    global LAST_EXEC_NS
    if "nc" not in _CACHE:
        _CACHE["nc"] = _build_nc()
    nc = _CACHE["nc"]

    in_maps = _prep_host(**inputs)

    trace = bool(int(os.environ.get("KERNEL_TRACE", "0")))
    if trace:
        _install_ntff_shim()

    tdir = None
    if trace:
        tdir = os.environ.get("KERNEL_TRACE_DIR")
        if tdir:
            os.makedirs(tdir, exist_ok=True)
    res = run_bass_kernel_spmd(nc, in_maps, core_ids=list(range(CORES)),
                               trace=trace, tmpdir=tdir)
    LAST_EXEC_NS = res.exec_time_ns

    out = np.empty((B, OUT), np.float32)
    for c in range(CORES):
        oT = res.results[c]["outT"].reshape(OUTP, BS)
        out[c * BS:(c + 1) * BS] = oT[:OUT].T
    return out


# revision 21
# speedup vs baseline: 1.0493x; 1.0018x over previous
"""DendriticMLP Trainium2 kernel — 8-core hybrid data/tensor parallel.

v3 vs baseline (pure data-parallel): the dendrite path (89% of matmul
cycles, 402 MB/core of fp32r segment-weight streaming when replicated) is
tensor-sharded: core c computes d = ctx @ sw for unit quarter r = c % 4
(512 units) over batch group q = c // 4 (2048 rows). Segment-weight traffic
drops to ~120 MB/core (~170 MB total vs 433 MB), removing the HBM
co-binding that made replicated runs stochastically DMA-bound (baseline
measured 1.85-2.10 ms run-to-run; this version measured 1.862-1.871 ms
across 6 normal-clock runs, 1.944 ms on one slow-clock day — the spread is
platform clock weather, not DMA). The PE stream is unchanged: 6912 matmuls x 512
cols, fp32r dendrites (argmax selection precision), fp16 W-path. Measured
PE floor at the platform GPIO clock cap (13/16 = 1.95 GHz; engages under
sustained PE load regardless of DMA traffic — probed with a half-traffic
bf16 run): ~1.82 ms.

Gate exchange: one 8-rank AllToAll per (layer, unit-tile) piece (bypass,
1 MB). SPMD uniformity: a core cannot index A2A shards by its own group, so
each sender mirrors its 4 gate chunks into both groups' shard slots (every
shard holds valid sigmoids, no NaN) and each receiver picks between the two
candidate shards with copy_predicated on a per-core constant mask input.
Piece u fires ~135 us before piece u+1 so the ~30 us collective latency
hides under remaining dendrite matmuls; at layer boundaries a
COVER_S-segment chunk of the next layer's dendrites is emitted before the
W block to cover piece 3's flight.

Tail: the last layer's final gate piece would leave the classifier waiting
~45 us on a collective fired after the last dendrite matmul. Instead the
4 unit tiles of classifier piece 3 ({3,7,11,15}) are computed REPLICATED —
each core for its own 512-row batch shard, gates purely local (no A2A in
the tail). That swaps 512 sharded for 512 replicated matmuls (PE count
unchanged). The replicated piece streams its segment weights in fp16
(16.8 MB, keeps tail-phase HBM demand at ~120 GB/s); fp16 operand noise on
that quarter of the units raises rel-err 1.593e-2 -> 1.650e-2 (gate 2e-2,
measured, deterministic).
Classifier accumulates kt-split over 8 concurrent PSUM banks as each gate
piece lands.

BatchNorm stays exact full-batch via a 16 KB AllReduce of per-unit
(sum, sum_sq), overlapped with dendrite matmuls. Dendrite
argmax-|.|-gather is gather-free: running max/min over the 16 segment
outputs, then sel = where(max >= -min, max, min).
"""
import os
import sys
import types

sys.path.insert(0, "/opt/trn_rl_repo")

import numpy as np

import concourse.bass as bass
import concourse.mybir as mybir
import concourse.tile as tile
from concourse import bacc
from concourse.bass_utils import run_bass_kernel_spmd

B, D, H, S, OUT = 4096, 1024, 2048, 16, 1000
CORES = 8
BS = B // CORES            # 512 rows per core (W-path shard / A2A shard)
NQ = 4                     # unit quarters
GB = B // (CORES // NQ)    # 2048 rows per dendrite batch group
UQ = H // NQ               # 512 units per core for dendrites
UT_Q = UQ // 128           # 4 unit tiles per core for dendrites
OUTP = 1024                # classifier outputs padded to 8*128
KT_D = D // 128            # 8 k-tiles for 1024-dim contractions
KT_H = H // 128            # 16 k-tiles for 2048-dim contractions
UT_H = H // 128            # 16 unit tiles per hidden layer
UT_O = OUTP // 128         # 8 unit tiles for classifier
TAILK = [k for k in range(UT_H) if k % UT_Q == 3]   # {3,7,11,15}
BN_EPS = 1e-5

F32 = mybir.dt.float32
F32R = mybir.dt.float32r
F16 = mybir.dt.float16
AX = mybir.AxisListType
ALU = mybir.AluOpType
ACTF = mybir.ActivationFunctionType

LAST_EXEC_NS = None
_CACHE = {}
DBG_NO_COLL = bool(int(os.environ.get("DBG_NO_COLL", "0")))
COVER_S = int(os.environ.get("COVER_S", "8"))


def _install_ntff_shim():
    """Register antenv.axon_hooks so bass_utils can NTFF-profile under axon."""
    if "antenv.axon_hooks" in sys.modules:
        return
    try:
        from trn_agent_boot.trn_boot import _ntff_profile_via_ctypes

        hook = _ntff_profile_via_ctypes("/opt/axon/libaxon_pjrt.so")
        mod = types.ModuleType("antenv.axon_hooks")
        state = {"hook": hook}
        mod.set_axon_ntff_profile_hook = lambda h: state.__setitem__("hook", h)
        mod.get_axon_ntff_profile_hook = lambda: state["hook"]
        sys.modules["antenv.axon_hooks"] = mod
    except Exception:
        pass


def _build_nc():
    nc = bacc.Bacc("TRN2", target_bir_lowering=False, debug=False,
                   num_devices=CORES)

    xinT = nc.dram_tensor("xinT", [KT_D, 128, BS], F16, kind="ExternalInput").ap()
    ctxT = nc.dram_tensor("ctxT", [KT_D, 128, GB], F32R, kind="ExternalInput").ap()
    # own-batch-shard ctx for the replicated tail piece
    ctx2T = nc.dram_tensor("ctx2T", [KT_D, 128, BS], F16,
                           kind="ExternalInput").ap()
    wr0 = nc.dram_tensor("wr0", [UT_H, 128, KT_D, 128], F16, kind="ExternalInput").ap()
    wr1 = nc.dram_tensor("wr1", [UT_H, 128, KT_H, 128], F16, kind="ExternalInput").ap()
    wr2 = nc.dram_tensor("wr2", [UT_H, 128, KT_H, 128], F16, kind="ExternalInput").ap()
    # classifier weights, piece-major: [out_ut, piece u, 128ki, 4r*128u]
    wcr = nc.dram_tensor("wcr", [UT_O, UT_Q, 128, NQ * 128], F16,
                         kind="ExternalInput").ap()
    # sharded segment weights (unit quarter, layers 0-2; layer 2 only ut 0..2)
    swr = [
        nc.dram_tensor(f"swr{i}", [UT_Q, S, 128, KT_D, 128], F32R,
                       kind="ExternalInput").ap()
        for i in range(3)
    ]
    # replicated fp16 tail piece: layer-2 sw for global unit tiles {3,7,11,15}
    swtr = nc.dram_tensor("swtr", [NQ, S, 128, KT_D, 128], F16,
                          kind="ExternalInput").ap()
    br = nc.dram_tensor("br", [128, 3 * UT_H], F32, kind="ExternalInput").ap()
    bcr = nc.dram_tensor("bcr", [128, UT_O], F32, kind="ExternalInput").ap()
    gmr = nc.dram_tensor("gmr", [128, BS], mybir.dt.uint8,
                         kind="ExternalInput").ap()
    outT = nc.dram_tensor("outT", [UT_O, 128, BS], F32, kind="ExternalOutput").ap()

    wr = [wr0, wr1, wr2]

    with tile.TileContext(nc) as tc:
        with (
            tc.tile_pool(name="pers", bufs=1) as pers,
            tc.tile_pool(name="wblk", bufs=4) as wpool,
            tc.tile_pool(name="wcc", bufs=8) as wcpool,
            tc.tile_pool(name="swp", bufs=7) as swpool,
            tc.tile_pool(name="mxp", bufs=2) as mxpool,
            tc.tile_pool(name="work", bufs=1) as work,
            tc.tile_pool(name="gio", bufs=2) as gio,
            tc.tile_pool(name="ob", bufs=2) as opool,
            tc.tile_pool(name="pb", bufs=2, space="PSUM") as pb,
            tc.tile_pool(name="pd", bufs=6, space="PSUM") as pd,
            tc.tile_pool(name="dram", bufs=1, space="DRAM") as dram,
        ):
            # ---- persistent tiles ----
            # xf slots first hold xin (fp16, W0 moving operand), later the
            # replicated tail piece's own-shard ctx (f32r) — same tags, the
            # pool ring (bufs=1) reuses the slot with automatic deps.
            xin_tiles = [pers.tile([128, BS], F16, tag=f"xf{k}", name=f"xin{k}")
                         for k in range(KT_D)]
            ctx_tiles = [pers.tile([128, GB], F32R, tag=f"ctx{k}", name=f"ctx{k}")
                         for k in range(KT_D)]
            h_tiles = [pers.tile([128, BS], F16, tag=f"h{k}", name=f"h{k}")
                       for k in range(UT_H)]
            y_tiles = [pers.tile([128, BS], F16, tag=f"y{k}", name=f"y{k}")
                       for k in range(UT_H)]
            bias_sb = pers.tile([128, 3 * UT_H], F32, tag="bias_sb", name="bias_sb")
            bc_sb = pers.tile([128, UT_O], F32, tag="bc_sb", name="bc_sb")
            gmask = pers.tile([128, BS], mybir.dt.uint8, tag="gmask",
                              name="gmask")

            nc.sync.dma_start(bias_sb[:], br)
            for k in range(KT_D):
                nc.sync.dma_start(xin_tiles[k][:], xinT[k])

            gin = [[dram.tile([CORES, 128, BS], F16, tag=f"gi{l}_{u}",
                              name=f"gi{l}_{u}") for u in range(UT_Q)]
                   for l in range(3)]
            gout = [[dram.tile([CORES, 128, BS], F16, tag=f"go{l}_{u}",
                               name=f"go{l}_{u}") for u in range(UT_Q)]
                    for l in range(3)]

            bn = {}  # layer -> (scale, nbias)

            def emit_w_block(layer, interleave=None):
                kt_in = KT_D if layer == 0 else KT_H
                in_tiles = xin_tiles if layer == 0 else h_tiles
                stats_loc = pers.tile([128, 2 * UT_H], F32, tag=f"stl{layer}",
                                      name=f"stl{layer}")
                for ut in range(UT_H):
                    if interleave and ut in interleave:
                        interleave[ut]()
                    wchunk = wpool.tile([128, kt_in * 128], F16, tag="wblk",
                                        name=f"w{layer}_{ut}")
                    src = wr[layer][ut].rearrange("p a b -> p (a b)")
                    if layer == 0 and ut < 2:
                        # halves so the first matmuls start after 128 KB
                        nc.sync.dma_start(wchunk[:, :kt_in * 64], src[:, :kt_in * 64])
                        nc.sync.dma_start(wchunk[:, kt_in * 64:], src[:, kt_in * 64:])
                    else:
                        nc.sync.dma_start(wchunk[:], src)
                    ps = pb.tile([128, BS], F32, tag="yblk", name=f"yp{layer}_{ut}")
                    for kt in range(kt_in):
                        nc.tensor.matmul(
                            ps[:],
                            wchunk[:, kt * 128:(kt + 1) * 128],
                            in_tiles[kt][:],
                            start=(kt == 0),
                            stop=(kt == kt_in - 1),
                        )
                    y = y_tiles[ut]
                    nc.scalar.activation(
                        y[:], ps[:], ACTF.Identity,
                        bias=bias_sb[:, layer * UT_H + ut:layer * UT_H + ut + 1],
                    )
                    nc.vector.tensor_reduce(
                        stats_loc[:, ut:ut + 1], y[:], axis=AX.X, op=ALU.add)
                    sq = work.tile([128, BS], F32, tag="sq", name=f"sq{layer}_{ut}")
                    nc.scalar.activation(
                        sq[:], y[:], ACTF.Square,
                        accum_out=stats_loc[:, UT_H + ut:UT_H + ut + 1],
                    )
                stats_glob = pers.tile([128, 2 * UT_H], F32, tag=f"stg{layer}",
                                       name=f"stg{layer}")
                if DBG_NO_COLL:
                    nc.vector.tensor_scalar_mul(stats_glob[:], stats_loc[:],
                                                float(CORES))
                else:
                    bnc_in = dram.tile([128, 2 * UT_H], F32, tag=f"bin{layer}",
                                       name=f"bin{layer}")
                    bnc_out = dram.tile([128, 2 * UT_H], F32, addr_space="Shared",
                                        tag=f"bout{layer}", name=f"bout{layer}")
                    nc.sync.dma_start(bnc_in[:], stats_loc[:])
                    nc.gpsimd.collective_compute(
                        "AllReduce", ALU.add,
                        ins=[bnc_in.opt()],
                        outs=[bnc_out.opt()],
                        replica_groups=[list(range(CORES))],
                    )
                    nc.sync.dma_start(stats_glob[:], bnc_out[:])
                return stats_glob

            def emit_bn_coeffs(layer, stats_glob):
                mean = pers.tile([128, UT_H], F32, tag=f"mean{layer}",
                                 name=f"mean{layer}")
                var = pers.tile([128, UT_H], F32, tag=f"var{layer}",
                                name=f"var{layer}")
                scale = pers.tile([128, UT_H], F32, tag=f"scale{layer}",
                                  name=f"scale{layer}")
                nbias = pers.tile([128, UT_H], F32, tag=f"nbias{layer}",
                                  name=f"nbias{layer}")
                msq = pers.tile([128, UT_H], F32, tag=f"msq{layer}",
                                name=f"msq{layer}")
                nc.vector.tensor_scalar_mul(mean[:], stats_glob[:, 0:UT_H], 1.0 / B)
                nc.vector.tensor_scalar_mul(var[:], stats_glob[:, UT_H:2 * UT_H],
                                            1.0 / B)
                nc.vector.scalar_tensor_tensor(
                    out=msq[:], in0=mean[:], scalar=-1.0, in1=mean[:],
                    op0=ALU.mult, op1=ALU.mult,
                )
                nc.vector.tensor_tensor(var[:], var[:], msq[:], op=ALU.add)
                nc.vector.tensor_scalar_add(var[:], var[:], BN_EPS)
                nc.scalar.sqrt(scale[:], var[:])
                nc.vector.reciprocal(scale[:], scale[:])
                nc.vector.scalar_tensor_tensor(
                    out=nbias[:], in0=mean[:], scalar=-1.0, in1=scale[:],
                    op0=ALU.mult, op1=ALU.mult,
                )
                bn[layer] = (scale, nbias)

            def alloc_mxmn(nm):
                mx = [mxpool.tile([128, BS], F32, tag=f"mx{bc}",
                                  name=f"mx_{nm}_{bc}") for bc in range(4)]
                mn = [mxpool.tile([128, BS], F32, tag=f"mn{bc}",
                                  name=f"mn_{nm}_{bc}") for bc in range(4)]
                return mx, mn

            def emit_dend_piece(layer, ut, s_range, mx, mn, pre=None):
                for s in s_range:
                    if pre is not None and s < len(pre):
                        swc = pre[s]
                    else:
                        swc = swpool.tile([128, KT_D * 128], F32R, tag="sw",
                                          name=f"sw{layer}_{ut}_{s}")
                        nc.sync.dma_start(
                            swc[:],
                            swr[layer][ut, s].rearrange("p a b -> p (a b)"),
                        )
                    for bc in range(4):
                        psd = pd.tile([128, BS], F32, tag="pd",
                                      name=f"pd{layer}_{ut}_{s}_{bc}")
                        for kt in range(KT_D):
                            nc.tensor.matmul(
                                psd[:],
                                swc[:, kt * 128:(kt + 1) * 128],
                                ctx_tiles[kt][:, bc * BS:(bc + 1) * BS],
                                start=(kt == 0),
                                stop=(kt == KT_D - 1),
                            )
                        if s == 0:
                            nc.scalar.copy(mx[bc][:], psd[:])
                            nc.vector.tensor_copy(mn[bc][:], psd[:])
                        else:
                            nc.vector.tensor_tensor(mx[bc][:], mx[bc][:], psd[:],
                                                    op=ALU.max)
                            nc.vector.tensor_tensor(mn[bc][:], mn[bc][:], psd[:],
                                                    op=ALU.min)

            def emit_dend_gates(layer, ut, mx, mn):
                """sel = where(mx >= -mn, mx, mn); sigmoid; mirrored shard
                writes; fire the A2A piece."""
                for bc in range(4):
                    negmn = work.tile([128, BS], F32, tag="negmn",
                                      name=f"ng{layer}_{ut}_{bc}")
                    nc.scalar.mul(negmn[:], mn[bc][:], -1.0)
                    mask = work.tile([128, BS], mybir.dt.uint8, tag="mask",
                                     name=f"mk{layer}_{ut}_{bc}")
                    nc.vector.tensor_tensor(mask[:], mx[bc][:], negmn[:],
                                            op=ALU.is_ge)
                    nc.vector.copy_predicated(mn[bc][:], mask[:], mx[bc][:])
                    gs = gio.tile([128, BS], F16, tag="gs",
                                  name=f"gs{layer}_{ut}_{bc}")
                    nc.scalar.activation(gs[:], mn[bc][:], ACTF.Sigmoid)
                    nc.sync.dma_start(gin[layer][ut][bc], gs[:])
                    nc.sync.dma_start(gin[layer][ut][bc + 4], gs[:])
                if not DBG_NO_COLL:
                    nc.gpsimd.collective_compute(
                        "AllToAll", ALU.bypass,
                        ins=[gin[layer][ut].opt()],
                        outs=[gout[layer][ut].opt()],
                        replica_groups=[list(range(CORES))],
                    )

            def emit_h_tiles(layer, ks):
                """h[k] = relu(bn(y[k])) * select(ga, gb)."""
                scale, nbias = bn[layer]
                for k in ks:
                    u = k % UT_Q
                    r = k // UT_Q
                    src = gout[layer][u]
                    if DBG_NO_COLL:
                        src = gin[layer][u]
                    ga = gio.tile([128, BS], F16, tag="ga", name=f"ga{layer}_{k}")
                    gb = gio.tile([128, BS], F16, tag="gb", name=f"gb{layer}_{k}")
                    nc.sync.dma_start(ga[:], src[r])
                    nc.sync.dma_start(gb[:], src[r + 4])
                    # group-0 cores keep ga (shard r); group-1 take gb (r+4)
                    nc.vector.copy_predicated(ga[:], gmask[:], gb[:])
                    nc.scalar.activation(
                        h_tiles[k][:], y_tiles[k][:], ACTF.Relu,
                        bias=nbias[:, k:k + 1], scale=scale[:, k:k + 1],
                    )
                    nc.vector.tensor_tensor(h_tiles[k][:], h_tiles[k][:], ga[:],
                                            op=ALU.mult)

            def emit_tail_piece():
                """Layer-2 dendrites for global unit tiles {3,7,11,15},
                replicated per core over its own 512-row batch shard (fp16
                sw), gates local — no collective in the tail."""
                scale, nbias = bn[2]
                ctx2 = _ctx2
                for t in range(NQ):
                    k = TAILK[t]
                    mx = mxpool.tile([128, BS], F32, tag=f"mx{t}", name=f"mxT{t}")
                    mn = mxpool.tile([128, BS], F32, tag=f"mn{t}", name=f"mnT{t}")
                    for s in range(S):
                        swc = swpool.tile([128, KT_D * 128], F16, tag="sw",
                                          name=f"swt{t}_{s}")
                        nc.sync.dma_start(
                            swc[:],
                            swtr[t, s].rearrange("p a b -> p (a b)"),
                        )
                        psd = pd.tile([128, BS], F32, tag="pd",
                                      name=f"pdt{t}_{s}")
                        for kt in range(KT_D):
                            nc.tensor.matmul(
                                psd[:],
                                swc[:, kt * 128:(kt + 1) * 128],
                                ctx2[kt][:],
                                start=(kt == 0),
                                stop=(kt == KT_D - 1),
                            )
                        if s == 0:
                            nc.scalar.copy(mx[:], psd[:])
                            nc.vector.tensor_copy(mn[:], psd[:])
                        else:
                            nc.vector.tensor_tensor(mx[:], mx[:], psd[:],
                                                    op=ALU.max)
                            nc.vector.tensor_tensor(mn[:], mn[:], psd[:],
                                                    op=ALU.min)
                    negmn = work.tile([128, BS], F32, tag="negmn", name=f"ngT{t}")
                    nc.scalar.mul(negmn[:], mn[:], -1.0)
                    mask = work.tile([128, BS], mybir.dt.uint8, tag="mask",
                                     name=f"mkT{t}")
                    nc.vector.tensor_tensor(mask[:], mx[:], negmn[:], op=ALU.is_ge)
                    nc.vector.copy_predicated(mn[:], mask[:], mx[:])
                    gs = gio.tile([128, BS], F16, tag="gs", name=f"gsT{t}")
                    nc.scalar.activation(gs[:], mn[:], ACTF.Sigmoid)
                    nc.scalar.activation(
                        h_tiles[k][:], y_tiles[k][:], ACTF.Relu,
                        bias=nbias[:, k:k + 1], scale=scale[:, k:k + 1],
                    )
                    nc.vector.tensor_tensor(h_tiles[k][:], h_tiles[k][:], gs[:],
                                            op=ALU.mult)

            # ================= program =================
            _ctx2 = [pers.tile([128, BS], F16, tag=f"xf{k}", name=f"c2_{k}")
                     for k in range(KT_D)]
            pre_sw = [swpool.tile([128, KT_D * 128], F32R, tag="sw",
                                  name=f"swpre{s}") for s in range(3)]

            def _issue_pre_sw():
                for s2 in range(3):
                    nc.sync.dma_start(
                        pre_sw[s2][:],
                        swr[0][0, s2].rearrange("p a b -> p (a b)"))

            def _issue_ctx():
                # quarter-major: D0's s=0 sweeps batch chunks in order, and
                # chunk bc needs columns [bc*512,(bc+1)*512) of ALL 8 kt —
                # issuing quarter-by-quarter across kt makes the first matmul
                # group's 2.1 MB arrive first instead of last
                for qtr in range(4):
                    a, b = qtr * (GB // 4), (qtr + 1) * (GB // 4)
                    for k in range(KT_D):
                        nc.sync.dma_start(ctx_tiles[k][:, a:b], ctxT[k, :, a:b])

            def _issue_late_small():
                nc.sync.dma_start(bc_sb[:], bcr)
                nc.sync.dma_start(gmask[:], gmr)

            # CC-channel warmup: the first collective of a NEFF costs ~68 us
            # (cold channel + peer arrival skew); pay it at t=0 under full
            # PE cover so the BN AllReduces and gate AllToAlls run warm.
            wci_sb = pers.tile([128, 4], F32, tag="wci", name="wci")
            nc.vector.memset(wci_sb[:], 0.0)
            wcc_in = dram.tile([128, 4], F32, tag="wcc_in", name="wcc_in")
            wcc_out = dram.tile([128, 4], F32, addr_space="Shared",
                                tag="wcc_out", name="wcc_out")
            nc.sync.dma_start(wcc_in[:], wci_sb[:])
            if not DBG_NO_COLL:
                nc.gpsimd.collective_compute(
                    "AllReduce", ALU.add,
                    ins=[wcc_in.opt()], outs=[wcc_out.opt()],
                    replica_groups=[list(range(CORES))],
                )

            # PE clock warmup: the HAM/GPIO un-throttle needs ~3.4 us of
            # sustained PE busy; the first real matmul waits ~17 us for DMA.
            # Dummy matmuls on zeroed scratch (never read) bridge the gap so
            # W0 starts at full clock instead of 1.2 GHz.
            warm_s = pers.tile([128, 128], F16, tag="warm_s", name="warm_s")
            warm_m = pers.tile([128, BS], F16, tag="warm_m", name="warm_m")
            nc.vector.memset(warm_s[:], 0.0)
            nc.vector.memset(warm_m[:], 0.0)
            # length tuned to end just before the earliest DMA-ready time
            # (~16 us): the residual <3.4 us idle keeps HAM warm without the
            # dummy chain ever delaying the first real matmul.
            wps = pb.tile([128, BS], F32, tag="yblk", name="warmps")
            for _ in range(100):
                nc.tensor.matmul(wps[:, :128], warm_s[:], warm_m[:, :128],
                                 start=True, stop=True)

            stats = {0: emit_w_block(0, interleave={2: _issue_pre_sw,
                                                    4: _issue_ctx,
                                                    7: _issue_late_small})}

            carry = None  # (mx, mn) of the next layer's piece-0 cover chunk
            for layer in range(3):
                n_shard = UT_Q if layer < 2 else UT_Q - 1
                for ut in range(n_shard):
                    if ut == 0 and carry is not None:
                        mx, mn = carry
                        emit_dend_piece(layer, 0, range(COVER_S, S), mx, mn)
                        carry = None
                    else:
                        mx, mn = alloc_mxmn(f"{layer}_{ut}")
                        emit_dend_piece(layer, ut, range(S), mx, mn,
                                        pre=pre_sw if (layer, ut) == (0, 0)
                                        else None)
                    emit_dend_gates(layer, ut, mx, mn)

                if layer < 2:
                    ks_early = [k for k in range(UT_H) if k % UT_Q < 3]
                    ks_late = [k for k in range(UT_H) if k % UT_Q == 3]
                    emit_bn_coeffs(layer, stats[layer])
                    emit_h_tiles(layer, ks_early)
                    cmx, cmn = alloc_mxmn(f"c{layer + 1}")
                    emit_dend_piece(layer + 1, 0, range(COVER_S), cmx, cmn)
                    emit_h_tiles(layer, ks_late)
                    stats[layer + 1] = emit_w_block(layer + 1)
                    carry = (cmx, cmn)
                    if layer == 0:
                        # tail-piece ctx: xf slots are free once W0 read xin;
                        # emitting here lets the DMA run ~1 ms early
                        for k in range(KT_D):
                            nc.sync.dma_start(_ctx2[k][:], ctx2T[k])

            # layer-2 replicated tail piece (gates local, h for {3,7,11,15})
            emit_bn_coeffs(2, stats[2])
            emit_tail_piece()

            # ---- classifier (kt-split accumulation over 8 PSUM banks) ----
            cps = [(pd if i < 6 else pb).tile([128, BS], F32,
                                              tag=("pd" if i < 6 else "yblk"),
                                              name=f"cps{i}")
                   for i in range(UT_O)]
            for u in range(UT_Q):
                ks = [r * UT_Q + u for r in range(NQ)]
                if u < 3:
                    emit_h_tiles(2, ks)   # u==3 h tiles came from the tail piece
                for o in range(UT_O):
                    wcc = wcpool.tile([128, NQ * 128], F16, tag="wcc",
                                      name=f"wcc{u}_{o}")
                    nc.sync.dma_start(wcc[:], wcr[o, u])
                    for j, k in enumerate(ks):
                        nc.tensor.matmul(
                            cps[o][:],
                            wcc[:, j * 128:(j + 1) * 128],
                            h_tiles[k][:],
                            start=(u == 0 and j == 0),
                            stop=(u == UT_Q - 1 and j == NQ - 1),
                        )
                    if u == UT_Q - 1:
                        osb = opool.tile([128, BS], F32, tag="osb",
                                         name=f"osb{o}")
                        nc.scalar.activation(osb[:], cps[o][:], ACTF.Identity,
                                             bias=bc_sb[:, o:o + 1])
                        nc.sync.dma_start(outT[o], osb[:])

    nc.compile()
    return nc


def _prep_host(x, w0, b0, sw0, w1, b1, sw1, w2, b2, sw2, wc, bc):
    f = np.float32
    h16 = np.float16

    def _w_reorder(w, kt):  # w [H_out, K] -> [ut, 128ki, kt, 128u]
        wT = np.ascontiguousarray(w.astype(h16).T)        # [K, H_out]
        K, HO = wT.shape
        return np.ascontiguousarray(
            wT.reshape(kt, 128, HO // 128, 128).transpose(2, 1, 0, 3))

    wc_pad = np.zeros((OUTP, H), f)
    wc_pad[:OUT] = wc.astype(f)
    bc_pad = np.zeros((OUTP,), f)
    bc_pad[:OUT] = bc.astype(f)

    # classifier: [out_ut, 128ki, kt=16, 128u], kt = 4r+u
    #   -> [out_ut, piece u, 128ki, (r, 128u)]
    wcr = _w_reorder(wc_pad, KT_H)                        # [8, 128, 16, 128]
    wcr = np.ascontiguousarray(
        wcr.reshape(UT_O, 128, NQ, UT_Q, 128).transpose(0, 3, 1, 2, 4)
        .reshape(UT_O, UT_Q, 128, NQ * 128))

    def _sw_reorder(sl, dt):  # [U, S, D] -> [U/128, S, 128ki, kt, 128u]
        nt = sl.shape[0] // 128
        return np.ascontiguousarray(
            sl.astype(dt).reshape(nt, 128, S, KT_D, 128).transpose(0, 2, 4, 3, 1))

    # replicated fp16 tail piece: layer-2 tiles {3,7,11,15}
    swt = np.concatenate([sw2[k * 128:(k + 1) * 128] for k in TAILK], axis=0)

    sws = [sw0, sw1, sw2]
    common = {
        "wr0": _w_reorder(w0, KT_D),
        "wr1": _w_reorder(w1, KT_H),
        "wr2": _w_reorder(w2, KT_H),
        "wcr": wcr,
        "swtr": _sw_reorder(swt, h16),
        "br": np.ascontiguousarray(
            np.stack([b0, b1, b2]).astype(f).reshape(3 * UT_H, 128).T),
        "bcr": np.ascontiguousarray(bc_pad.reshape(UT_O, 128).T),
    }
    in_maps = []
    for c in range(CORES):
        q, r = c // NQ, c % NQ
        xs = x[c * BS:(c + 1) * BS]
        xg = x[q * GB:(q + 1) * GB]
        m = dict(common)
        m["xinT"] = np.ascontiguousarray(
            xs[:, :D].astype(h16).T).reshape(KT_D, 128, BS)
        m["ctxT"] = np.ascontiguousarray(
            xg[:, D:].astype(f).T).reshape(KT_D, 128, GB)
        m["ctx2T"] = np.ascontiguousarray(
            xs[:, D:].astype(h16).T).reshape(KT_D, 128, BS)
        for i, sw in enumerate(sws):
            m[f"swr{i}"] = _sw_reorder(sw[r * UQ:(r + 1) * UQ], f)
        m["gmr"] = np.full((128, BS), q, np.uint8)
        in_maps.append(m)
    return in_maps


def kernel(**inputs):
    global LAST_EXEC_NS
    if "nc" not in _CACHE:
        _CACHE["nc"] = _build_nc()
    nc = _CACHE["nc"]

    in_maps = _prep_host(**inputs)

    trace = bool(int(os.environ.get("KERNEL_TRACE", "0")))
    if trace:
        _install_ntff_shim()

    tdir = None
    if trace:
        tdir = os.environ.get("KERNEL_TRACE_DIR")
        if tdir:
            os.makedirs(tdir, exist_ok=True)
    res = run_bass_kernel_spmd(nc, in_maps, core_ids=list(range(CORES)),
                               trace=trace, tmpdir=tdir)
    LAST_EXEC_NS = res.exec_time_ns

    out = np.empty((B, OUT), np.float32)
    for c in range(CORES):
        oT = res.results[c]["outT"].reshape(OUTP, BS)
        out[c * BS:(c + 1) * BS] = oT[:OUT].T
    return out
